# revision 6
# baseline (speedup 1.0000x reference)
"""2D DCT-II (ortho) on (32, 3, 512, 512) fp32, data-parallel across 8 TRN2 NeuronCores.

v2: quad-fold + bf16 matmuls. Both axes are folded using the DCT symmetry
D[k, 511-n] = (-1)^k D[k, n]:
  S = X[n', :] + X[511-n', :],  Dd = X[n', :] - X[511-n', :]     (n' < 256, H fold)
  EE/EO = S[:, w'] +/- S[:, 511-w'],  OE/OO = Dd +/- Dd_rev      (w' < 256, W fold)
Each quadrant Q (parity ph, pw) then needs only 256-length contractions with a
single 256x256 constant C_p[a, b] = D[2b + p, a] (identical matrix for both
passes):
  pass A: P1_Q[w', k'] = sum_n' Q[n', w'] C_ph[n', k']
  pass B: OUT[2k'+ph, 2kw'+pw] = sum_w' P1_Q[w', k'] C_pw[w', kw']
PE work: 8192 MM rows/image (vs 12288 unfolded-W baseline), all matmuls bf16
256-free.  The H-fold pairing (n', 511-n') is arranged at load time: the upper
image half is pulled with the 128-row block dim outermost (positive step) and a
negative row step at dim 1 (dim-0 steps must be positive for the BIR verifier),
so partners share an SBUF partition; the resulting block swap (hi block c
pairs with lo chunk 1-c) is undone with a negative free-dim AP in the S/D ops.
Pass B writes even/odd kw interleaved into PSUM via stride-2 output APs, so
the PSUM->SBUF copy and the store are fully contiguous (no shuffle pass).
Engine split per image: PE 32 MM + 32 LDW; gpsimd S/Dd + OE/OO folds; DVE
EE/EO folds + 2 out-copies; ACT p1 copies + 2 out-copies + store dispatch;
sync input loads.
"""
import os
import sys

for _p in ("/opt/trn_rl_repo", os.path.expanduser("~/.axon_site/_ro/trn_rl_repo")):
    if os.path.isdir(_p) and _p not in sys.path:
        sys.path.insert(0, _p)

import numpy as np
from ml_dtypes import bfloat16
import concourse.bass as bass
import concourse.bacc as bacc
import concourse.mybir as mybir
import concourse.tile as tile
from concourse.bass_utils import run_bass_kernel_spmd

dt = mybir.dt

N = 512            # image height/width
H = N // 2         # 256, folded length
P = 128            # SBUF partitions
N_CORES = 8
B, CH = 32, 3      # full input batch/channels
IMGS = (B * CH) // N_CORES  # 12 images per core


def _dct_matrix() -> np.ndarray:
    n = np.arange(N, dtype=np.float64)
    k = n[:, None]
    D = np.cos(np.pi * (2.0 * n[None, :] + 1.0) * k / (2.0 * N))
    D[0] *= np.sqrt(1.0 / N)
    D[1:] *= np.sqrt(2.0 / N)
    return D


def _consts() -> np.ndarray:
    D = _dct_matrix()
    ce = D[0::2, :H].T  # [a, b] = D[2b, a]
    co = D[1::2, :H].T  # [a, b] = D[2b+1, a]
    return np.concatenate([ce, co], axis=0).astype(bfloat16)  # [512, 256]


def _build_nc() -> bacc.Bacc:
    nc = bacc.Bacc("TRN2", target_bir_lowering=False, debug=False, num_devices=N_CORES)
    inp = nc.dram_tensor("inp", [IMGS, N, N], dt.float32, kind="ExternalInput")
    out = nc.dram_tensor("out", [IMGS, N, N], dt.float32, kind="ExternalOutput")
    cmat = nc.dram_tensor("cmat", [2 * H, H], dt.bfloat16, kind="ExternalInput")

    f32 = dt.float32
    bf = dt.bfloat16
    QUADS = ["ee", "eo", "oe", "oo"]  # (ph, pw) parities

    with tile.TileContext(nc) as tc:
        with (
            tc.tile_pool(name="const", bufs=1) as const_pool,
            tc.tile_pool(name="xin", bufs=3) as xin_pool,
            tc.tile_pool(name="sd", bufs=2) as sd_pool,
            tc.tile_pool(name="quad", bufs=2) as quad_pool,
            tc.tile_pool(name="p1", bufs=2) as p1_pool,
            tc.tile_pool(name="res", bufs=2) as res_pool,
            tc.tile_pool(name="psa", bufs=1, space="PSUM") as psa_pool,
            tc.tile_pool(name="psb", bufs=1, space="PSUM") as psb_pool,
        ):
            # C constants: cmat_sb[p, g*H + b] = C[p + 128*(g%2), b] with
            # g in {0,1}: Ce chunks, g in {2,3}: Co chunks (rows a = p + 128g mod 256).
            cmat_sb = const_pool.tile([P, 4 * H], bf)
            nc.scalar.dma_start(
                cmat_sb[:].rearrange("p (g b) -> p g b", g=4),
                cmat.ap().rearrange("(g p) b -> p g b", p=P),
            )

            def rhs(par: int, c: int):  # par 0=even,1=odd ; c = contraction chunk
                g = 2 * par + c
                return cmat_sb[:, g * H : (g + 1) * H]

            # PE warmup during the initial DMA ramp (HAM clock gate to 8/8).
            scr_f = const_pool.tile([P, H + P], f32)
            nc.gpsimd.memset(scr_f[:], 0.0)
            scr = const_pool.tile([P, H + P], bf)
            nc.vector.tensor_copy(scr[:], scr_f[:])
            ps_w = psb_pool.tile([P, N], f32, tag="psB_e0")
            for _ in range(8):
                nc.tensor.matmul(
                    ps_w[:, 0:H], scr[:, H : H + P], scr[:, 0:H], start=True, stop=True
                )

            for i in range(IMGS):
                ia = inp.ap()[i]

                # --- loads ---
                # lo[p, c*N + w] = X[p + 128c, w]
                xlo_t = xin_pool.tile([P, 2 * N], f32, tag="xlo")
                nc.sync.dma_start(
                    xlo_t[:].rearrange("p (c f) -> p c f", c=2),
                    bass.AP(ia.tensor, ia.offset, [[N, P], [P * N, 2], [1, N]]),
                )
                # hi[p, c*N + w] = X[511 - p - 128c, w]: the host pre-permutes
                # rows 256.. of "inp" to hold X's mirrored rows in ascending
                # order (see run()), so this is a plain ascending load.
                xhi_t = xin_pool.tile([P, 2 * N], f32, tag="xhi")
                nc.sync.dma_start(
                    xhi_t[:].rearrange("p (c f) -> p c f", c=2),
                    bass.AP(ia.tensor, ia.offset + 2 * P * N, [[N, P], [P * N, 2], [1, N]]),
                )

                # --- H fold: S = lo + hi, Dd = lo - hi (gpsimd, contiguous) ---
                s_full = sd_pool.tile([P, 2 * N], f32, tag="S")
                d_full = sd_pool.tile([P, 2 * N], f32, tag="D")
                nc.gpsimd.tensor_add(s_full[:], xlo_t[:], xhi_t[:])
                nc.gpsimd.tensor_sub(d_full[:], xlo_t[:], xhi_t[:])

                # --- W fold -> 4 bf16 quadrants (EE/EO on DVE, OE/OO on gpsimd) ---
                # quad tile cols: c*H + j  (j = w' in [0,H))
                qt = {q: quad_pool.tile([P, 2 * H], bf, tag=q, name=f"{q}_{i}") for q in QUADS}

                def qslice(q, c, t):
                    return qt[q][:, c * H + t * P : c * H + (t + 1) * P]

                for q, src, eng in (
                    ("ee", s_full, nc.vector.tensor_add),
                    ("eo", s_full, nc.vector.tensor_sub),
                    ("oe", d_full, nc.gpsimd.tensor_add),
                    ("oo", d_full, nc.gpsimd.tensor_sub),
                ):
                    sa = src[:]
                    lo_ap = bass.AP(sa.tensor, sa.offset, [[sa.ap[0][0], P], [N, 2], [1, H]])
                    hi_ap = bass.AP(
                        sa.tensor, sa.offset + N - 1, [[sa.ap[0][0], P], [N, 2], [-1, H]]
                    )
                    eng(qt[q][:].rearrange("p (c j) -> p c j", c=2), lo_ap, hi_ap)

                # --- pass A per quadrant: P1_Q[w', k'] ---
                p1 = {}

                def pass_a(q):
                    ph = 0 if q[0] == "e" else 1
                    ps = psa_pool.tile([P, 2 * H], f32, tag=f"psA_{q}")
                    for t in range(2):
                        for c in range(2):
                            nc.tensor.matmul(
                                ps[:, t * H : (t + 1) * H],
                                qslice(q, c, t),
                                rhs(ph, c),
                                start=(c == 0),
                                stop=(c == 1),
                            )
                    p1[q] = p1_pool.tile([P, 2 * H], bf, tag=f"p1_{q}", name=f"p1_{q}_{i}")
                    nc.scalar.copy(p1[q][:], ps[:])

                # --- pass B per ph: OUT rows 2k'+ph, kw interleaved in PSUM ---
                def pass_b(ph):
                    phs = "e" if ph == 0 else "o"
                    for m in range(2):
                        ps = psb_pool.tile([P, N], f32, tag=f"psB_{phs}{m}")
                        pb = ps[:]
                        for pw in range(2):
                            dst = bass.AP(
                                pb.tensor, pb.offset + pw, [[pb.ap[0][0], P], [2, H]]
                            )
                            q = phs + ("e" if pw == 0 else "o")
                            for t in range(2):
                                nc.tensor.matmul(
                                    dst,
                                    p1[q][:, t * H + m * P : t * H + (m + 1) * P],
                                    rhs(pw, t),
                                    start=(t == 0),
                                    stop=(t == 1),
                                )
                        o_sb = res_pool.tile([P, N], f32, tag=f"o_{phs}{m}")
                        cp = nc.vector.tensor_copy if m == 0 else nc.scalar.copy
                        cp(o_sb[:], ps[:])
                        # store rows 2*(m*128 + p) + ph
                        oa = out.ap()[i]
                        nc.scalar.dma_start(
                            bass.AP(
                                oa.tensor,
                                oa.offset + (2 * m * P + ph) * N,
                                [[2 * N, P], [1, N]],
                            ),
                            o_sb[:],
                        )

                # PE order keeps pass B fed while later pass As' copies land
                pass_a("ee")
                pass_a("eo")
                pass_a("oe")
                pass_b(0)
                pass_a("oo")
                pass_b(1)

    nc.compile()
    return nc


_NC_CACHE: bacc.Bacc | None = None


def _get_nc() -> bacc.Bacc:
    global _NC_CACHE
    if _NC_CACHE is None:
        _NC_CACHE = _build_nc()
    return _NC_CACHE


def run(inp: np.ndarray, **spmd_kwargs):
    """Shard, run on 8 cores, gather. Returns (output, BassKernelResults)."""
    x = np.asarray(inp, dtype=np.float32)
    assert x.shape == (B, CH, N, N), x.shape
    shards = x.reshape(N_CORES, IMGS, N, N)
    cm = _consts()
    # rows 256+p+128c hold X[511-p-128c]: mirrored partners land on the same
    # SBUF partition under plain ascending loads (dim-0 DMA steps must be >0).
    perm = np.concatenate(
        [np.arange(2 * P), 511 - np.arange(P), 383 - np.arange(P)]
    )
    in_maps = [
        {"inp": np.ascontiguousarray(shards[c][:, perm]), "cmat": cm}
        for c in range(N_CORES)
    ]
    res = run_bass_kernel_spmd(_get_nc(), in_maps, core_ids=list(range(N_CORES)), **spmd_kwargs)
    out = np.stack([res.results[c]["out"] for c in range(N_CORES)])
    return out.reshape(B, CH, N, N), res


def kernel(inp: np.ndarray) -> np.ndarray:
    out, _ = run(inp)
    return out


# revision 10
# speedup vs baseline: 1.2022x; 1.2022x over previous
"""2D DCT-II (ortho) on (32, 3, 512, 512) fp32, data-parallel across 8 TRN2 NeuronCores.

v2: quad-fold + bf16 matmuls. Both axes are folded using the DCT symmetry
D[k, 511-n] = (-1)^k D[k, n]:
  S = X[n', :] + X[511-n', :],  Dd = X[n', :] - X[511-n', :]     (n' < 256, H fold)
  EE/EO = S[:, w'] +/- S[:, 511-w'],  OE/OO = Dd +/- Dd_rev      (w' < 256, W fold)
Each quadrant Q (parity ph, pw) then needs only 256-length contractions with a
single 256x256 constant C_p[a, b] = D[2b + p, a] (identical matrix for both
passes):
  pass A: P1_Q[w', k'] = sum_n' Q[n', w'] C_ph[n', k']
  pass B: OUT[2k'+ph, 2kw'+pw] = sum_w' P1_Q[w', k'] C_pw[w', kw']
PE work: 8192 MM rows/image (vs 12288 unfolded-W baseline), all matmuls bf16
256-free.  The H-fold pairing (n', 511-n') is arranged at load time: the upper
image half is pulled with the 128-row block dim outermost (positive step) and a
negative row step at dim 1 (dim-0 steps must be positive for the BIR verifier),
so partners share an SBUF partition; the resulting block swap (hi block c
pairs with lo chunk 1-c) is undone with a negative free-dim AP in the S/D ops.
Pass B writes even/odd kw interleaved into PSUM via stride-2 output APs, so
the PSUM->SBUF copy and the store are fully contiguous (no shuffle pass).
Engine split per image: PE 32 MM + 32 LDW; gpsimd S/Dd + OE/OO folds; DVE
EE/EO folds + 2 out-copies; ACT p1 copies + 2 out-copies + store dispatch;
sync input loads.
"""
import os
import sys

for _p in ("/opt/trn_rl_repo", os.path.expanduser("~/.axon_site/_ro/trn_rl_repo")):
    if os.path.isdir(_p) and _p not in sys.path:
        sys.path.insert(0, _p)

import numpy as np
import concourse.bass as bass
import concourse.bacc as bacc
import concourse.mybir as mybir
import concourse.tile as tile
from concourse.bass_utils import run_bass_kernel_spmd

dt = mybir.dt

N = 512            # image height/width
H = N // 2         # 256, folded length
P = 128            # SBUF partitions
N_CORES = 8
B, CH = 32, 3      # full input batch/channels
IMGS = (B * CH) // N_CORES  # 12 images per core


def _dct_matrix() -> np.ndarray:
    n = np.arange(N, dtype=np.float64)
    k = n[:, None]
    D = np.cos(np.pi * (2.0 * n[None, :] + 1.0) * k / (2.0 * N))
    D[0] *= np.sqrt(1.0 / N)
    D[1:] *= np.sqrt(2.0 / N)
    return D


def _consts() -> np.ndarray:
    D = _dct_matrix()
    ce = D[0::2, :H].T  # [a, b] = D[2b, a]
    co = D[1::2, :H].T  # [a, b] = D[2b+1, a]
    return np.concatenate([ce, co], axis=0).astype(np.float32)  # [512, 256]


def _build_nc() -> bacc.Bacc:
    nc = bacc.Bacc("TRN2", target_bir_lowering=False, debug=False, num_devices=N_CORES)
    inp = nc.dram_tensor("inp", [IMGS, N, N], dt.float32, kind="ExternalInput")
    out = nc.dram_tensor("out", [IMGS, N, N], dt.float32, kind="ExternalOutput")
    cmat = nc.dram_tensor("cmat", [2 * H, H], dt.float32r, kind="ExternalInput")

    f32 = dt.float32
    fr = dt.float32r
    QUADS = ["ee", "eo", "oe", "oo"]  # (ph, pw) parities

    with tile.TileContext(nc) as tc:
        with (
            tc.tile_pool(name="const", bufs=1) as const_pool,
            tc.tile_pool(name="xin", bufs=3) as xin_pool,
            tc.tile_pool(name="sd", bufs=2) as sd_pool,
            tc.tile_pool(name="quad", bufs=2) as quad_pool,
            tc.tile_pool(name="p1", bufs=2) as p1_pool,
            tc.tile_pool(name="res", bufs=2) as res_pool,
            tc.tile_pool(name="psa", bufs=1, space="PSUM") as psa_pool,
            tc.tile_pool(name="psb", bufs=1, space="PSUM") as psb_pool,
        ):
            # C constants: cmat_sb[p, g*H + b] = C[p + 128*(g%2), b] with
            # g in {0,1}: Ce chunks, g in {2,3}: Co chunks (rows a = p + 128g mod 256).
            cmat_sb = const_pool.tile([P, 4 * H], fr)
            nc.scalar.dma_start(
                cmat_sb[:].rearrange("p (g b) -> p g b", g=4),
                cmat.ap().rearrange("(g p) b -> p g b", p=P),
            )

            def rhs(par: int, c: int):  # par 0=even,1=odd ; c = contraction chunk
                g = 2 * par + c
                return cmat_sb[:, g * H : (g + 1) * H]

            # bf16 copy of C for pass B (stride-2 PSUM matmul output is only
            # ISA-legal for 16-bit operand dtypes)
            cmat_bf = const_pool.tile([P, 4 * H], dt.bfloat16)
            nc.scalar.copy(cmat_bf[:], cmat_sb[:])

            def rhs_b(par: int, c: int):
                g = 2 * par + c
                return cmat_bf[:, g * H : (g + 1) * H]

            # PE warmup during the initial DMA ramp (HAM clock gate to 8/8).
            scr_f = const_pool.tile([P, H + P], f32)
            nc.gpsimd.memset(scr_f[:], 0.0)
            scr = const_pool.tile([P, H + P], fr)
            nc.vector.tensor_copy(scr[:], scr_f[:])
            ps_w = psb_pool.tile([P, N], f32, tag="psB_e0")
            for _ in range(8):
                nc.tensor.matmul(
                    ps_w[:, 0:H], scr[:, H : H + P], scr[:, 0:H], start=True, stop=True
                )

            for i in range(IMGS):
                ia = inp.ap()[i]

                # --- loads ---
                # lo[p, c*N + w] = X[p + 128c, w]
                xlo_t = xin_pool.tile([P, 2 * N], f32, tag="xlo")
                nc.sync.dma_start(
                    xlo_t[:].rearrange("p (c f) -> p c f", c=2),
                    bass.AP(ia.tensor, ia.offset, [[N, P], [P * N, 2], [1, N]]),
                )
                # hi[p, c*N + w] = X[511 - p - 128c, w]: the host pre-permutes
                # rows 256.. of "inp" to hold X's mirrored rows in ascending
                # order (see run()), so this is a plain ascending load.
                xhi_t = xin_pool.tile([P, 2 * N], f32, tag="xhi")
                nc.sync.dma_start(
                    xhi_t[:].rearrange("p (c f) -> p c f", c=2),
                    bass.AP(ia.tensor, ia.offset + 2 * P * N, [[N, P], [P * N, 2], [1, N]]),
                )

                # --- H fold: S = lo + hi, Dd = lo - hi (gpsimd, contiguous) ---
                s_full = sd_pool.tile([P, 2 * N], f32, tag="S")
                d_full = sd_pool.tile([P, 2 * N], f32, tag="D")
                nc.vector.tensor_add(s_full[:], xlo_t[:], xhi_t[:])
                nc.vector.tensor_sub(d_full[:], xlo_t[:], xhi_t[:])

                # --- W fold -> 4 bf16 quadrants (EE/EO on DVE, OE/OO on gpsimd) ---
                # quad tile cols: c*H + j  (j = w' in [0,H))
                qt = {q: quad_pool.tile([P, 2 * H], fr, tag=q, name=f"{q}_{i}") for q in QUADS}

                def qslice(q, c, t):
                    return qt[q][:, c * H + t * P : c * H + (t + 1) * P]

                for q, src, eng in (
                    ("ee", s_full, nc.vector.tensor_add),
                    ("eo", s_full, nc.vector.tensor_sub),
                    ("oe", d_full, nc.gpsimd.tensor_add),
                    ("oo", d_full, nc.gpsimd.tensor_sub),
                ):
                    sa = src[:]
                    lo_ap = bass.AP(sa.tensor, sa.offset, [[sa.ap[0][0], P], [N, 2], [1, H]])
                    hi_ap = bass.AP(
                        sa.tensor, sa.offset + N - 1, [[sa.ap[0][0], P], [N, 2], [-1, H]]
                    )
                    eng(qt[q][:].rearrange("p (c j) -> p c j", c=2), lo_ap, hi_ap)

                # --- pass A per quadrant: P1_Q[w', k'] ---
                p1 = {}

                def pass_a(q):
                    ph = 0 if q[0] == "e" else 1
                    ps = psa_pool.tile([P, 2 * H], f32, tag=f"psA_{q}")
                    for t in range(2):
                        for c in range(2):
                            nc.tensor.matmul(
                                ps[:, t * H : (t + 1) * H],
                                qslice(q, c, t),
                                rhs(ph, c),
                                start=(c == 0),
                                stop=(c == 1),
                            )
                    p1[q] = p1_pool.tile([P, 2 * H], dt.bfloat16, tag=f"p1_{q}", name=f"p1_{q}_{i}")
                    nc.scalar.copy(p1[q][:], ps[:])

                # --- pass B per ph: OUT rows 2k'+ph, kw interleaved in PSUM ---
                def pass_b(ph):
                    phs = "e" if ph == 0 else "o"
                    o_sb = res_pool.tile([P, 2 * N], f32, tag=f"o_{phs}", name=f"o_{phs}_{i}")
                    for m in range(2):
                        ps = psb_pool.tile([P, N], f32, tag=f"psB_{phs}{m}")
                        pb = ps[:]
                        for pw in range(2):
                            dst = bass.AP(
                                pb.tensor, pb.offset + pw, [[pb.ap[0][0], P], [2, H]]
                            )
                            q = phs + ("e" if pw == 0 else "o")
                            for t in range(2):
                                nc.tensor.matmul(
                                    dst,
                                    p1[q][:, t * H + m * P : t * H + (m + 1) * P],
                                    rhs_b(pw, t),
                                    start=(t == 0),
                                    stop=(t == 1),
                                )
                        cp = nc.vector.tensor_copy if m == 0 else nc.scalar.copy
                        cp(o_sb[:, m * N : (m + 1) * N], ps[:])
                    # one store per parity: rows 2*(m*128 + p) + ph
                    oa = out.ap()[i]
                    nc.sync.dma_start(
                        bass.AP(
                            oa.tensor,
                            oa.offset + ph * N,
                            [[2 * N, P], [2 * P * N, 2], [1, N]],
                        ),
                        o_sb[:].rearrange("p (m f) -> p m f", m=2),
                    )

                # PE order keeps pass B fed while later pass As' copies land
                pass_a("ee")
                pass_a("eo")
                pass_a("oe")
                pass_b(0)
                pass_a("oo")
                pass_b(1)

    nc.compile()
    return nc


_NC_CACHE: bacc.Bacc | None = None


def _get_nc() -> bacc.Bacc:
    global _NC_CACHE
    if _NC_CACHE is None:
        _NC_CACHE = _build_nc()
    return _NC_CACHE


def run(inp: np.ndarray, **spmd_kwargs):
    """Shard, run on 8 cores, gather. Returns (output, BassKernelResults)."""
    x = np.asarray(inp, dtype=np.float32)
    assert x.shape == (B, CH, N, N), x.shape
    shards = x.reshape(N_CORES, IMGS, N, N)
    cm = _consts()
    # rows 256+p+128c hold X[511-p-128c]: mirrored partners land on the same
    # SBUF partition under plain ascending loads (dim-0 DMA steps must be >0).
    perm = np.concatenate(
        [np.arange(2 * P), 511 - np.arange(P), 383 - np.arange(P)]
    )
    in_maps = [
        {"inp": np.ascontiguousarray(shards[c][:, perm]), "cmat": cm}
        for c in range(N_CORES)
    ]
    res = run_bass_kernel_spmd(_get_nc(), in_maps, core_ids=list(range(N_CORES)), **spmd_kwargs)
    out = np.stack([res.results[c]["out"] for c in range(N_CORES)])
    return out.reshape(B, CH, N, N), res


def kernel(inp: np.ndarray) -> np.ndarray:
    out, _ = run(inp)
    return out


# revision 12
# speedup vs baseline: 1.2353x; 1.0275x over previous
"""2D DCT-II (ortho) on (32, 3, 512, 512) fp32, data-parallel across 8 TRN2 NeuronCores.

v2: quad-fold + bf16 matmuls. Both axes are folded using the DCT symmetry
D[k, 511-n] = (-1)^k D[k, n]:
  S = X[n', :] + X[511-n', :],  Dd = X[n', :] - X[511-n', :]     (n' < 256, H fold)
  EE/EO = S[:, w'] +/- S[:, 511-w'],  OE/OO = Dd +/- Dd_rev      (w' < 256, W fold)
Each quadrant Q (parity ph, pw) then needs only 256-length contractions with a
single 256x256 constant C_p[a, b] = D[2b + p, a] (identical matrix for both
passes):
  pass A: P1_Q[w', k'] = sum_n' Q[n', w'] C_ph[n', k']
  pass B: OUT[2k'+ph, 2kw'+pw] = sum_w' P1_Q[w', k'] C_pw[w', kw']
PE work: 8192 MM rows/image (vs 12288 unfolded-W baseline), all matmuls bf16
256-free.  The H-fold pairing (n', 511-n') is arranged at load time: the upper
image half is pulled with the 128-row block dim outermost (positive step) and a
negative row step at dim 1 (dim-0 steps must be positive for the BIR verifier),
so partners share an SBUF partition; the resulting block swap (hi block c
pairs with lo chunk 1-c) is undone with a negative free-dim AP in the S/D ops.
Pass B writes even/odd kw interleaved into PSUM via stride-2 output APs, so
the PSUM->SBUF copy and the store are fully contiguous (no shuffle pass).
Engine split per image: PE 32 MM + 32 LDW; gpsimd S/Dd + OE/OO folds; DVE
EE/EO folds + 2 out-copies; ACT p1 copies + 2 out-copies + store dispatch;
sync input loads.
"""
import os
import sys

for _p in ("/opt/trn_rl_repo", os.path.expanduser("~/.axon_site/_ro/trn_rl_repo")):
    if os.path.isdir(_p) and _p not in sys.path:
        sys.path.insert(0, _p)

import numpy as np
import concourse.bass as bass
import concourse.bacc as bacc
import concourse.mybir as mybir
import concourse.tile as tile
from concourse.bass_utils import run_bass_kernel_spmd

dt = mybir.dt

N = 512            # image height/width
H = N // 2         # 256, folded length
P = 128            # SBUF partitions
N_CORES = 8
B, CH = 32, 3      # full input batch/channels
IMGS = (B * CH) // N_CORES  # 12 images per core


def _dct_matrix() -> np.ndarray:
    n = np.arange(N, dtype=np.float64)
    k = n[:, None]
    D = np.cos(np.pi * (2.0 * n[None, :] + 1.0) * k / (2.0 * N))
    D[0] *= np.sqrt(1.0 / N)
    D[1:] *= np.sqrt(2.0 / N)
    return D


def _consts() -> np.ndarray:
    D = _dct_matrix()
    ce = D[0::2, :H].T  # [a, b] = D[2b, a]
    co = D[1::2, :H].T  # [a, b] = D[2b+1, a]
    return np.concatenate([ce, co], axis=0).astype(np.float32)  # [512, 256]


def _build_nc() -> bacc.Bacc:
    nc = bacc.Bacc("TRN2", target_bir_lowering=False, debug=False, num_devices=N_CORES)
    inp = nc.dram_tensor("inp", [IMGS, N, N], dt.float32, kind="ExternalInput")
    out = nc.dram_tensor("out", [IMGS, N, N], dt.float32, kind="ExternalOutput")
    cmat = nc.dram_tensor("cmat", [2 * H, H], dt.float32r, kind="ExternalInput")

    f32 = dt.float32
    fr = dt.float32r
    QUADS = ["ee", "eo", "oe", "oo"]  # (ph, pw) parities

    with tile.TileContext(nc) as tc:
        with (
            tc.tile_pool(name="const", bufs=1) as const_pool,
            tc.tile_pool(name="xin", bufs=3) as xin_pool,
            tc.tile_pool(name="sd", bufs=2) as sd_pool,
            tc.tile_pool(name="quad", bufs=2) as quad_pool,
            tc.tile_pool(name="p1", bufs=2) as p1_pool,
            tc.tile_pool(name="res", bufs=2) as res_pool,
            tc.tile_pool(name="psa", bufs=1, space="PSUM") as psa_pool,
            tc.tile_pool(name="psb", bufs=1, space="PSUM") as psb_pool,
        ):
            # C constants: cmat_sb[p, g*H + b] = C[p + 128*(g%2), b] with
            # g in {0,1}: Ce chunks, g in {2,3}: Co chunks (rows a = p + 128g mod 256).
            cmat_sb = const_pool.tile([P, 4 * H], fr)
            nc.scalar.dma_start(
                cmat_sb[:].rearrange("p (g b) -> p g b", g=4),
                cmat.ap().rearrange("(g p) b -> p g b", p=P),
            )

            def rhs(par: int, c: int):  # par 0=even,1=odd ; c = contraction chunk
                g = 2 * par + c
                return cmat_sb[:, g * H : (g + 1) * H]

            # bf16 copy of C for pass B (stride-2 PSUM matmul output is only
            # ISA-legal for 16-bit operand dtypes)
            cmat_bf = const_pool.tile([P, 4 * H], dt.bfloat16)
            nc.scalar.copy(cmat_bf[:], cmat_sb[:])

            def rhs_b(par: int, c: int):
                g = 2 * par + c
                return cmat_bf[:, g * H : (g + 1) * H]

            # PE warmup during the initial DMA ramp (HAM clock gate to 8/8).
            scr_f = const_pool.tile([P, H + P], f32)
            nc.gpsimd.memset(scr_f[:], 0.0)
            scr = const_pool.tile([P, H + P], fr)
            nc.vector.tensor_copy(scr[:], scr_f[:])
            ps_w = psb_pool.tile([P, 4 * H], f32, tag="psB_e")
            for _ in range(8):
                nc.tensor.matmul(
                    ps_w[:, 0:H], scr[:, H : H + P], scr[:, 0:H], start=True, stop=True
                )

            for i in range(IMGS):
                ia = inp.ap()[i]

                # --- loads ---
                # lo[p, c*N + w] = X[p + 128c, w]
                xlo_t = xin_pool.tile([P, 2 * N], f32, tag="xlo")
                nc.sync.dma_start(
                    xlo_t[:].rearrange("p (c f) -> p c f", c=2),
                    bass.AP(ia.tensor, ia.offset, [[N, P], [P * N, 2], [1, N]]),
                )
                # hi[p, c*N + w] = X[511 - p - 128c, w]: the host pre-permutes
                # rows 256.. of "inp" to hold X's mirrored rows in ascending
                # order (see run()), so this is a plain ascending load.
                xhi_t = xin_pool.tile([P, 2 * N], f32, tag="xhi")
                nc.sync.dma_start(
                    xhi_t[:].rearrange("p (c f) -> p c f", c=2),
                    bass.AP(ia.tensor, ia.offset + 2 * P * N, [[N, P], [P * N, 2], [1, N]]),
                )

                # --- H fold: S = lo + hi, Dd = lo - hi (gpsimd, contiguous) ---
                s_full = sd_pool.tile([P, 2 * N], f32, tag="S")
                d_full = sd_pool.tile([P, 2 * N], f32, tag="D")
                nc.vector.tensor_add(s_full[:], xlo_t[:], xhi_t[:])
                nc.vector.tensor_sub(d_full[:], xlo_t[:], xhi_t[:])

                # --- W fold -> 4 bf16 quadrants (EE/EO on DVE, OE/OO on gpsimd) ---
                # quad tile cols: c*H + j  (j = w' in [0,H))
                qt = {q: quad_pool.tile([P, 2 * H], fr, tag=q, name=f"{q}_{i}") for q in QUADS}

                def qslice(q, c, t):
                    return qt[q][:, c * H + t * P : c * H + (t + 1) * P]

                for q, src, eng in (
                    ("ee", s_full, nc.vector.tensor_add),
                    ("eo", s_full, nc.vector.tensor_sub),
                    ("oe", d_full, nc.gpsimd.tensor_add),
                    ("oo", d_full, nc.gpsimd.tensor_sub),
                ):
                    sa = src[:]
                    lo_ap = bass.AP(sa.tensor, sa.offset, [[sa.ap[0][0], P], [N, 2], [1, H]])
                    hi_ap = bass.AP(
                        sa.tensor, sa.offset + N - 1, [[sa.ap[0][0], P], [N, 2], [-1, H]]
                    )
                    eng(qt[q][:].rearrange("p (c j) -> p c j", c=2), lo_ap, hi_ap)

                # --- pass A: quadrant pairs share a 2-bank PSUM tile so the
                # PSUM->SBUF copy is one big ACT op per parity ---
                p1 = {}

                def pass_a(ph):
                    phs = "e" if ph == 0 else "o"
                    ps = psa_pool.tile([P, 4 * H], f32, tag=f"psA_{phs}")
                    for pw in range(2):
                        q = phs + ("e" if pw == 0 else "o")
                        for t in range(2):
                            for c in range(2):
                                nc.tensor.matmul(
                                    ps[:, pw * N + t * H : pw * N + (t + 1) * H],
                                    qslice(q, c, t),
                                    rhs(ph, c),
                                    start=(c == 0),
                                    stop=(c == 1),
                                )
                    p1[phs] = p1_pool.tile([P, 4 * H], dt.bfloat16, tag=f"p1_{phs}", name=f"p1_{phs}_{i}")
                    nc.scalar.copy(p1[phs][:], ps[:])

                # --- pass B per ph: OUT rows 2k'+ph, kw interleaved in PSUM ---
                def pass_b(ph):
                    phs = "e" if ph == 0 else "o"
                    ps = psb_pool.tile([P, 4 * H], f32, tag=f"psB_{phs}")
                    pb = ps[:]
                    for m in range(2):
                        for pw in range(2):
                            dst = bass.AP(
                                pb.tensor, pb.offset + m * N + pw, [[pb.ap[0][0], P], [2, H]]
                            )
                            for t in range(2):
                                nc.tensor.matmul(
                                    dst,
                                    p1[phs][:, pw * N + t * H + m * P : pw * N + t * H + (m + 1) * P],
                                    rhs_b(pw, t),
                                    start=(t == 0),
                                    stop=(t == 1),
                                )
                    o_sb = res_pool.tile([P, 2 * N], f32, tag=f"o_{phs}", name=f"o_{phs}_{i}")
                    nc.scalar.copy(o_sb[:], ps[:])
                    # one store per parity: rows 2*(m*128 + p) + ph
                    oa = out.ap()[i]
                    nc.sync.dma_start(
                        bass.AP(
                            oa.tensor,
                            oa.offset + ph * N,
                            [[2 * N, P], [2 * P * N, 2], [1, N]],
                        ),
                        o_sb[:].rearrange("p (m f) -> p m f", m=2),
                    )

                # all pass A first: p1(o) copy overlaps pass B(e) on the PE
                pass_a(0)
                pass_a(1)
                pass_b(0)
                pass_b(1)

    nc.compile()
    return nc


_NC_CACHE: bacc.Bacc | None = None


def _get_nc() -> bacc.Bacc:
    global _NC_CACHE
    if _NC_CACHE is None:
        _NC_CACHE = _build_nc()
    return _NC_CACHE


def run(inp: np.ndarray, **spmd_kwargs):
    """Shard, run on 8 cores, gather. Returns (output, BassKernelResults)."""
    x = np.asarray(inp, dtype=np.float32)
    assert x.shape == (B, CH, N, N), x.shape
    shards = x.reshape(N_CORES, IMGS, N, N)
    cm = _consts()
    # rows 256+p+128c hold X[511-p-128c]: mirrored partners land on the same
    # SBUF partition under plain ascending loads (dim-0 DMA steps must be >0).
    perm = np.concatenate(
        [np.arange(2 * P), 511 - np.arange(P), 383 - np.arange(P)]
    )
    in_maps = [
        {"inp": np.ascontiguousarray(shards[c][:, perm]), "cmat": cm}
        for c in range(N_CORES)
    ]
    res = run_bass_kernel_spmd(_get_nc(), in_maps, core_ids=list(range(N_CORES)), **spmd_kwargs)
    out = np.stack([res.results[c]["out"] for c in range(N_CORES)])
    return out.reshape(B, CH, N, N), res


def kernel(inp: np.ndarray) -> np.ndarray:
    out, _ = run(inp)
    return out
